# revision 38
# baseline (speedup 1.0000x reference)
"""Trainium2 Bass kernel for the AdditiveModel reduction.

Computes out[y] = sum_{q,p} c[y,q] * a[y,q,p] * dot(lam[y,q,p,:], x[q,p,:])
with Y=16, Q=8, P=32, D=8192 (lam is 128 MiB -> memory-bound).

Sharding: one q per core (Q == 8 cores). Each core is fully independent and
produces a partial out[16]; the host sums the 8 partials at gather time.

Per-core compute: the D-axis dot products run on the TensorEngine. At
sharding time the host hands each core its lam slice pre-transposed to
[d, (y,p)] layout (d on partitions) and cast to fp16 (total output error
~1e-4 of output scale, comparable to the PE's fp32r path), so the dots
become 64 PSUM-accumulated matmuls lhsT=x[dchunk, p] (128x32),
rhs=lam[dchunk, (y,p)] (128x512) with fp32 accumulation. PSUM then holds
G[m, (y,p)] = dot(x[p_m,:], lam[y,p,:]); a single DVE multiply by a
weighted diagonal mask (maskW = diag-mask * c*a, built mid-stream with a
step-0 broadcast AP) plus a p-group reduce yields a [32, 16] per-core
partial whose 32-partition collapse happens in the host gather.

Streaming: lam is split in chunk-halves across the two HWDGE rings (SP and
ACT) so both hardware descriptor generators run in parallel; the matmul
order interleaves the two streams. Tile has only 8 DMA-completion semaphore
lanes, and a DMA whose lane is reused must wait for the earlier DMA on that
lane to COMPLETE before issuing -- so the small consts are packed into one
DMA and slab sizes are arranged so every lane reuse waits on an
early-finishing transfer. The last slabs are small: a slab's matmuls can
only start ~2.6us after its last byte (completion receipt), so the final
slab's consumption sits on the critical path. All slabs are
SBUF-resident; gpsimd SWDGE is avoided (its Q7 descriptor generation starts
~5us late).
"""

from contextlib import ExitStack

import numpy as np

Y, Q, P, D = 16, 8, 32, 8192
NCORES = 8
KC = 128                 # contraction chunk (partition count)
DC = D // KC             # 64 d-chunks
YP = Y * P               # 512
SLAB_CHUNKS = [5, 5, 5, 5, 4, 4, 2, 2]   # per-ring slab sizes (small tail)
WARMUP_MM = 0            # optional discarded PE matmuls before data lands
CMB_W = YP + Y + Y       # packed const width: m0 | aT | crep

_CACHE = {}


def _build_nc():
    import concourse.bass as bass
    import concourse.mybir as mybir
    import concourse.tile as tile
    from concourse import bacc

    f32 = mybir.dt.float32
    f16 = mybir.dt.float16
    nc = bacc.Bacc(None, target_bir_lowering=False)

    lamT = nc.declare_dram_parameter("lamT", [KC, DC * YP], f16, isOutput=False)
    xT = nc.declare_dram_parameter("xT", [KC, DC * P], f16, isOutput=False)
    cmb = nc.declare_dram_parameter("cmb", [P, CMB_W], f32, isOutput=False)
    out = nc.declare_dram_parameter("out", [P, Y], f32, isOutput=True)

    with tile.TileContext(nc) as tc, ExitStack() as ctx:
        const = ctx.enter_context(tc.tile_pool(name="const", bufs=1))
        slab_pool = ctx.enter_context(
            tc.tile_pool(name="slab", bufs=len(SLAB_CHUNKS))
        )
        psum_pool = ctx.enter_context(
            tc.tile_pool(name="psum", bufs=1, space=bass.MemorySpace.PSUM)
        )
        tailp = ctx.enter_context(tc.tile_pool(name="tail", bufs=1))

        # x halves load first on both rings: they gate the first matmuls.
        x_sb = const.tile([KC, DC * P], f16)
        xh = DC // 2 * P
        nc.sync.dma_start(x_sb[:, 0:xh], xT[:, 0:xh])
        nc.scalar.dma_start(x_sb[:, xh:2 * xh], xT[:, xh:2 * xh])

        # PE warm-up: data-independent matmuls on a memset tile so the HAM
        # activity monitor unthrottles the PE clock before real data lands.
        if WARMUP_MM:
            warm = const.tile([KC, YP], f16)
            nc.gpsimd.memset(warm[:], 0.0)
            wpsum = psum_pool.tile([P, YP], f32, name="wpsum")
            for _ in range(WARMUP_MM):
                nc.tensor.matmul(
                    wpsum[:], warm[:, 0:P], warm[:], start=True, stop=True
                )

        proj = psum_pool.tile([P, YP], f32)
        half = DC // 2
        assert sum(SLAB_CHUNKS) == half
        mm_seq = []
        lo = 0
        for s, cps in enumerate(SLAB_CHUNKS):
            slab_a = slab_pool.tile([KC, cps * YP], f16, tag="slab_a")
            a_lo = lo
            nc.sync.dma_start(slab_a[:], lamT[:, a_lo * YP:(a_lo + cps) * YP])
            slab_b = slab_pool.tile([KC, cps * YP], f16, tag="slab_b")
            b_lo = half + lo
            nc.scalar.dma_start(slab_b[:], lamT[:, b_lo * YP:(b_lo + cps) * YP])
            for c in range(cps):
                mm_seq.append((a_lo + c, slab_a[:, c * YP:(c + 1) * YP]))
                mm_seq.append((b_lo + c, slab_b[:, c * YP:(c + 1) * YP]))
            lo += cps

        for i, (cg, ap) in enumerate(mm_seq):
            nc.tensor.matmul(
                proj[:],
                x_sb[:, cg * P:(cg + 1) * P],
                ap,
                start=(i == 0),
                stop=(i == len(mm_seq) - 1),
            )

        # packed consts (m0 | aT | crep) in ONE DMA on the ACT ring
        cmb_sb = const.tile([P, CMB_W], f32)
        nc.scalar.dma_start(cmb_sb[:], cmb[:])
        m0_sb = cmb_sb[:, 0:YP]
        aT_sb = cmb_sb[:, YP:YP + Y]
        cr_sb = cmb_sb[:, YP + Y:YP + 2 * Y]
        wT = const.tile([P, Y], f32)
        nc.vector.tensor_mul(wT[:], aT_sb, cr_sb)
        # fold the (c*a) weights into the diag mask mid-stream (off the
        # critical path): maskW[m, (y,p)] = m0 * w[y,m]
        maskW = const.tile([P, YP], f32)
        nc.vector.tensor_mul(
            maskW[:].rearrange("m (y p) -> m y p", p=P),
            m0_sb.rearrange("m (y p) -> m y p", p=P),
            wT[:].rearrange("m (y o) -> m y o", o=1).broadcast_to([P, Y, P]),
        )

        # tail: masked+weighted copy of PSUM, p-group reduce, DMA out.
        # The final 32-partition collapse happens host-side at gather time.
        t2 = tailp.tile([P, YP], f32)
        nc.vector.tensor_mul(t2[:], proj[:], maskW[:])
        s_t = tailp.tile([P, Y], f32)
        nc.vector.reduce_sum(
            s_t[:],
            t2[:].rearrange("m (y p) -> m y p", p=P),
            axis=mybir.AxisListType.X,
        )
        nc.scalar.dma_start(out[:], s_t[:])

    nc.compile()
    return nc


def _shard_inputs(x, lam, a, c):
    """Per-core input maps. Slicing/layout/dtype transforms only."""
    m0_np = np.tile(np.eye(P, dtype=np.float32), (1, Y))          # [P, Y*P]
    in_maps = []
    for q in range(NCORES):
        lam_q = lam[:, q]                                          # [Y, P, D]
        lamT = np.ascontiguousarray(
            lam_q.transpose(2, 0, 1).reshape(DC, KC, YP)
            .transpose(1, 0, 2).reshape(KC, DC * YP)
        )
        x_q = x[q]                                                 # [P, D]
        xTn = np.ascontiguousarray(
            x_q.T.reshape(DC, KC, P).transpose(1, 0, 2).reshape(KC, DC * P)
        )
        aTn = a[:, q].T.astype(np.float32)                         # [P, Y]
        crn = np.broadcast_to(c[:, q][None, :], (P, Y)).astype(np.float32)
        cmb_np = np.ascontiguousarray(
            np.concatenate([m0_np, aTn, crn], axis=1)
        )
        in_maps.append(
            {
                "lamT": lamT.astype(np.float16),
                "xT": xTn.astype(np.float16),
                "cmb": cmb_np,
            }
        )
    return in_maps


def get_nc():
    key = (tuple(SLAB_CHUNKS), WARMUP_MM)
    if key not in _CACHE:
        _CACHE[key] = _build_nc()
    return _CACHE[key]


def run(x, lam, a, c, trace=False, **spmd_kwargs):
    from concourse.bass_utils import run_bass_kernel_spmd

    nc = get_nc()
    in_maps = _shard_inputs(
        np.asarray(x, dtype=np.float32),
        np.asarray(lam, dtype=np.float32),
        np.asarray(a, dtype=np.float32),
        np.asarray(c, dtype=np.float32),
    )
    res = run_bass_kernel_spmd(
        nc, in_maps, core_ids=list(range(NCORES)), trace=trace, **spmd_kwargs
    )
    out = np.zeros((Y,), dtype=np.float32)
    for core_res in res.results:
        out += core_res["out"].reshape(P, Y).sum(axis=0)
    return out, res


def kernel(x, lam, a, c):
    try:
        out, _ = run(x, lam, a, c, trace=False)
    except Exception:
        # one retry to ride out transient device errors
        out, _ = run(x, lam, a, c, trace=False)
    return out
